# revision 11
# baseline (speedup 1.0000x reference)
"""Causal single-head attention (B=4, S=2048, D=1024, fp32) on 8 TRN2 cores.

Reference computation (per batch b):
    scores = (x @ qk) @ x.T / sqrt(D)   causal-masked, softmax over keys
    out    = softmax(scores) @ x @ ov

Sharding: 2 cores per batch. Each core owns 8 of the 16 128-row query
blocks, snake-assigned ({4k, 4k+3} vs {4k+1, 4k+2}) so both halves see an
identical causal work profile -> one SPMD program, per-core data only.

Per-core pipeline (all matmuls in float32r, PSUM fp32 accumulation):
  xS = x_rows.T (PE transposes)        qT = qk.T @ xS
  xT = x_full.T (PE transposes)
  per query block i: scores strips = qT_i.T @ xT  (+additive causal mask
    from host), exp via ACT (accum_out = row-sums), PE-transpose probs
    into attnT
  per strip: PT = x_full.T-contracted attn (lhsT = natural x tiles)
  out block = (PT_i).T @ ov, scaled by 1/rowsum during PSUM evacuation.
"""

import numpy as np

import concourse.bacc as bacc
import concourse.bass as bass
import concourse.mybir as mybir
import concourse.tile as tile
from concourse.bass_interp import get_hw_module
from concourse.bass_utils import run_bass_kernel_spmd
from concourse.masks import make_identity

B, S, D = 4, 2048, 1024
NB = S // 128          # 16 row blocks per batch
NBL = NB // 2          # 8 row blocks per core
N_CORES = 8
SCALE = float(np.sqrt(D))
NEG = -1.0e30

# local block -> global block, per half (snake: exactly balanced causal work)
HALF_BLOCKS = [
    [0, 3, 4, 7, 8, 11, 12, 15],
    [1, 2, 5, 6, 9, 10, 13, 14],
]
# 512-wide score strips per local block (same for both halves)
CI = [1, 1, 2, 2, 3, 3, 4, 4]
E_STRIP = [8, 16]      # t-chunks accumulated per PT strip

F32 = mybir.dt.float32
F32R = mybir.dt.float32r


def _emit(nc, tc, x_full, x_rows, qk_in, ov_in, masks_in, y_out, ctx):
    f32r = lambda ap: ap.bitcast(F32R)
    DC = D // 128  # 8

    const = ctx.enter_context(tc.tile_pool(name="const", bufs=1))
    psA = ctx.enter_context(tc.tile_pool(name="psA", bufs=3, space="PSUM"))
    psT = ctx.enter_context(tc.tile_pool(name="psT", bufs=3, space="PSUM"))
    psP = ctx.enter_context(tc.tile_pool(name="psP", bufs=2, space="PSUM"))

    ident = const.tile([128, 128], F32, name="ident")
    make_identity(nc, ident)
    masks_sb = const.tile([128, NBL, 512], F32, name="masks_sb")
    nc.sync.dma_start(out=masks_sb, in_=masks_in.rearrange("i p t -> p i t"))
    zeros_sb = const.tile([128, 512], F32, name="zeros_sb")
    nc.vector.memset(zeros_sb, 0.0)
    recips = const.tile([128, NBL], F32, name="recips")

    attnT = [None, None]
    at_pool = ctx.enter_context(tc.tile_pool(name="attnT", bufs=1))
    attnT[0] = at_pool.tile([128, E_STRIP[0], 512], F32R, name="attnT0")
    attnT[1] = at_pool.tile([128, E_STRIP[1], 512], F32R, name="attnT1")
    # zero the never-written tail chunks (blocks whose causal extent ends
    # before the strip's max extent)
    for si, blkcol, lo in ((0, 0, 4), (0, 1, 4), (1, 0, 12), (1, 1, 12)):
        nc.vector.tensor_copy(
            attnT[si][:, lo:lo + 4, blkcol * 128:(blkcol + 1) * 128],
            zeros_sb.rearrange("p (a b) -> p a b", a=4),
        )

    qT_pool = tc.tile_pool(name="qT", bufs=1)
    qTp = qT_pool.__enter__()
    qT = qTp.tile([128, DC, 1024], F32R, name="qT")

    # ---- phase 1a: xS = x_rows.T ; qT = qk.T @ xS ----
    with tc.tile_pool(name="p1a", bufs=1) as p1a, \
         tc.tile_pool(name="xin_a", bufs=4) as xin_a:
        xS = p1a.tile([128, DC, 1024], F32R, name="xS")
        for rbg in range(2):          # groups of 4 row blocks
            xts = []
            for j in range(4):
                xt = xin_a.tile([128, 1024], F32, name="xta", tag="xta")
                nc.sync.dma_start(
                    out=xt, in_=x_rows[(rbg * 4 + j) * 128:(rbg * 4 + j + 1) * 128, :])
                xts.append(xt)
            for dc in range(DC):
                ps = psT.tile([128, 512], F32, name="ps_tr", tag="psT")
                for j in range(4):
                    nc.tensor.transpose(
                        ps[:, j * 128:(j + 1) * 128],
                        xts[j][:, dc * 128:(dc + 1) * 128], ident)
                nc.vector.tensor_copy(
                    xS[:, dc, rbg * 512:(rbg + 1) * 512], ps)

        qk_sb = p1a.tile([128, DC, 1024], F32R, name="qk_sb")
        nc.sync.dma_start(
            out=qk_sb, in_=f32r(qk_in.rearrange("(c p) e -> p c e", p=128)))
        for ec in range(DC):
            for st in range(2):
                ps = psA.tile([128, 512], F32, name="ps_qt", tag="psA")
                for dc in range(DC):
                    nc.tensor.matmul(
                        ps, qk_sb[:, dc, ec * 128:(ec + 1) * 128],
                        xS[:, dc, st * 512:(st + 1) * 512],
                        start=(dc == 0), stop=(dc == DC - 1))
                nc.vector.tensor_copy(qT[:, ec, st * 512:(st + 1) * 512], ps)

    # ---- phase 1b: xT = x_full.T ----
    xT_pool = tc.tile_pool(name="xT", bufs=1)
    xTp = xT_pool.__enter__()
    xT = xTp.tile([128, DC, S], F32R, name="xT")
    with tc.tile_pool(name="xin_b", bufs=4) as xin_b:
        for tbg in range(4):          # groups of 4 key blocks
            xts = []
            for j in range(4):
                xt = xin_b.tile([128, 1024], F32, name="xtb", tag="xtb")
                nc.sync.dma_start(
                    out=xt, in_=x_full[(tbg * 4 + j) * 128:(tbg * 4 + j + 1) * 128, :])
                xts.append(xt)
            for dc in range(DC):
                ps = psT.tile([128, 512], F32, name="ps_tr", tag="psT")
                for j in range(4):
                    nc.tensor.transpose(
                        ps[:, j * 128:(j + 1) * 128],
                        xts[j][:, dc * 128:(dc + 1) * 128], ident)
                nc.vector.tensor_copy(
                    xT[:, dc, tbg * 512:(tbg + 1) * 512], ps)

    # ---- phase 2: scores -> exp -> attnT, per local block ----
    with tc.tile_pool(name="p2", bufs=2) as p2, \
         tc.tile_pool(name="p2s", bufs=4) as p2s:
        for i in range(NBL):
            c = CI[i]
            si = i // 4
            blkcol = i % 4
            p = p2.tile([128, 2048], F32, name="p_probs", tag="p_probs")
            rs = p2s.tile([128, 4], F32, name="rs", tag="rs")
            for st in range(c):
                ps = psA.tile([128, 512], F32, name="ps_sc", tag="psA")
                for ec in range(DC):
                    nc.tensor.matmul(
                        ps, qT[:, ec, i * 128:(i + 1) * 128],
                        xT[:, ec, st * 512:(st + 1) * 512],
                        start=(ec == 0), stop=(ec == DC - 1))
                if st == c - 1:
                    nc.vector.tensor_add(ps, ps, masks_sb[:, i, :])
                nc.scalar.activation(
                    p[:, st * 512:(st + 1) * 512], ps,
                    mybir.ActivationFunctionType.Exp,
                    scale=1.0 / SCALE, accum_out=rs[:, st:st + 1])
            rsum = p2s.tile([128, 1], F32, name="rsum", tag="rsum")
            nc.vector.reduce_sum(rsum, rs[:, 0:c], axis=mybir.AxisListType.X)
            nc.vector.reciprocal(recips[:, i:i + 1], rsum)
            for st in range(c):
                ps = psT.tile([128, 512], F32, name="ps_at", tag="psT")
                for j in range(4):
                    tc_idx = st * 4 + j
                    nc.tensor.transpose(
                        ps[:, j * 128:(j + 1) * 128],
                        p[:, tc_idx * 128:(tc_idx + 1) * 128], ident)
                nc.vector.tensor_copy(
                    attnT[si][:, st * 4:st * 4 + 4,
                              blkcol * 128:(blkcol + 1) * 128],
                    ps.rearrange("p (a b) -> p a b", a=4))

    xT_pool.__exit__(None, None, None)
    qT_pool.__exit__(None, None, None)

    # ---- phase 3: PT = (attn @ x).T per strip; out = PT.T @ ov ----
    with tc.tile_pool(name="p3", bufs=1) as p3, \
         tc.tile_pool(name="xn_p", bufs=8) as xn_p, \
         tc.tile_pool(name="y_p", bufs=2) as y_p:
        ov_sb = p3.tile([128, DC, 1024], F32R, name="ov_sb")
        PT = p3.tile([128, DC, 1024], F32R, name="PT")
        # fp32 strip-scratch accumulator: PSUM can't hold 8 concurrent
        # d-chunk accumulators, so accumulate groups of 4 t-chunks in PSUM
        # and fold into SBUF
        PT32 = p3.tile([128, DC, 512], F32, name="PT32")
        ov_loaded = False
        for si in range(2):
            E = E_STRIP[si]
            for tcg in range(E // 4):
                xns = []
                for j in range(4):
                    tc_idx = tcg * 4 + j
                    xn = xn_p.tile([128, 1024], F32R, name="xn", tag="xn")
                    nc.sync.dma_start(
                        out=xn,
                        in_=f32r(x_full[tc_idx * 128:(tc_idx + 1) * 128, :]))
                    xns.append(xn)
                if not ov_loaded:
                    # emit after the first xn group so the 4MB ov transfer
                    # doesn't delay the PT-phase xn stream in the DMA queue
                    nc.sync.dma_start(
                        out=ov_sb,
                        in_=f32r(ov_in.rearrange("(c p) e -> p c e", p=128)))
                    ov_loaded = True
                for dc in range(DC):
                    ps = psP.tile([128, 512], F32, name="ps_pt", tag="psP")
                    for j in range(4):
                        nc.tensor.matmul(
                            ps, xns[j][:, dc * 128:(dc + 1) * 128],
                            attnT[si][:, tcg * 4 + j, :],
                            start=(j == 0), stop=(j == 3))
                    if tcg == 0:
                        nc.vector.tensor_copy(PT32[:, dc, :], ps)
                    else:
                        nc.vector.tensor_add(PT32[:, dc, :], PT32[:, dc, :], ps)
            for dc in range(DC):
                nc.vector.tensor_copy(
                    PT[:, dc, si * 512:(si + 1) * 512], PT32[:, dc, :])
            for bi in range(4):
                i = si * 4 + bi
                y_sb = y_p.tile([128, 1024], F32, name="y_sb", tag="y_sb")
                for es in range(2):
                    ps = psA.tile([128, 512], F32, name="ps_o", tag="psA")
                    for dc in range(DC):
                        nc.tensor.matmul(
                            ps, PT[:, dc, i * 128:(i + 1) * 128],
                            ov_sb[:, dc, es * 512:(es + 1) * 512],
                            start=(dc == 0), stop=(dc == DC - 1))
                    nc.scalar.activation(
                        y_sb[:, es * 512:(es + 1) * 512], ps,
                        mybir.ActivationFunctionType.Copy,
                        scale=recips[:, i:i + 1])
                nc.sync.dma_start(
                    out=y_out[i * 128:(i + 1) * 128, :], in_=y_sb)


_BUILT = {}


def _build(n_reps=1, timing=False):
    """timing=True builds a variant whose big tensors are Internal DRAM
    (garbage data, tiny external IO) so per-call transfer overhead over the
    axon tunnel doesn't swamp wall-clock differencing."""
    key = (n_reps, timing)
    if key in _BUILT:
        return _BUILT[key]
    from contextlib import ExitStack

    nc = bacc.Bacc(
        "TRN2", target_bir_lowering=False, debug=False,
        enable_asserts=False, num_devices=N_CORES)
    big = dict(kind="Internal") if timing else {}
    x_full = nc.dram_tensor("x_full", [S, D], F32,
                            **(big or dict(kind="ExternalInput"))).ap()
    x_rows = nc.dram_tensor("x_rows", [S // 2, D], F32,
                            **(big or dict(kind="ExternalInput"))).ap()
    qk_in = nc.dram_tensor("qk", [D, D], F32,
                           **(big or dict(kind="ExternalInput"))).ap()
    ov_in = nc.dram_tensor("ov", [D, D], F32,
                           **(big or dict(kind="ExternalInput"))).ap()
    masks_in = nc.dram_tensor(
        "masks", [NBL, 128, 512], F32, kind="ExternalInput").ap()
    y_out = nc.dram_tensor("y", [S // 2, D], F32,
                           **(big or dict(kind="ExternalOutput"))).ap()
    dummy_out = None
    if timing:
        dummy_out = nc.dram_tensor(
            "dummy_y", [128, 128], F32, kind="ExternalOutput").ap()

    with tile.TileContext(nc) as tc:
        if timing and n_reps > 1:
            with tc.For_i(0, n_reps, 1):
                with ExitStack() as ctx:
                    _emit(nc, tc, x_full, x_rows, qk_in, ov_in, masks_in,
                          y_out, ctx)
        else:
            for _ in range(n_reps):
                with ExitStack() as ctx:
                    _emit(nc, tc, x_full, x_rows, qk_in, ov_in, masks_in,
                          y_out, ctx)
        if timing:
            with tc.tile_pool(name="dummy_p", bufs=1) as dp:
                dt_ = dp.tile([128, 128], F32, name="dummy_sb")
                nc.sync.dma_start(out=dt_, in_=y_out[0:128, 0:128])
                nc.sync.dma_start(out=dummy_out, in_=dt_)
    nc.compile()
    nc.m = get_hw_module(nc.m)
    _BUILT[key] = nc
    return nc


def host_masks():
    masks = np.full((2, NBL, 128, 512), NEG, np.float32)
    for half in range(2):
        for i, g in enumerate(HALF_BLOCKS[half]):
            c = CI[i]
            t0 = 512 * (c - 1)
            t = t0 + np.arange(512)[None, :]
            s = 128 * g + np.arange(128)[:, None]
            masks[half, i] = np.where(t <= s, 0.0, NEG)
    return masks


def make_in_maps(input_data, qk, ov):
    x = np.ascontiguousarray(np.asarray(input_data, dtype=np.float32))
    qk = np.ascontiguousarray(np.asarray(qk, dtype=np.float32))
    ov = np.ascontiguousarray(np.asarray(ov, dtype=np.float32))
    masks = host_masks()
    in_maps = []
    for c in range(N_CORES):
        b, half = c // 2, c % 2
        rows = np.concatenate(
            [x[b, 128 * g:128 * (g + 1), :] for g in HALF_BLOCKS[half]], axis=0)
        in_maps.append({
            "x_full": x[b],
            "x_rows": np.ascontiguousarray(rows),
            "qk": qk,
            "ov": ov,
            "masks": np.ascontiguousarray(masks[half]),
        })
    return in_maps


def assemble(results):
    out = np.empty((B, S, D), np.float32)
    for c in range(N_CORES):
        b, half = c // 2, c % 2
        y = results[c]["y"]
        for i, g in enumerate(HALF_BLOCKS[half]):
            out[b, 128 * g:128 * (g + 1), :] = y[128 * i:128 * (i + 1), :]
    return out


def kernel(input_data, qk, ov):
    nc = _build()
    in_maps = make_in_maps(input_data, qk, ov)
    res = run_bass_kernel_spmd(nc, in_maps, core_ids=list(range(N_CORES)))
    return assemble(res.results)


# revision 31
# speedup vs baseline: 1.0093x; 1.0093x over previous
"""Causal single-head attention (B=4, S=2048, D=1024, fp32) on 8 TRN2 cores.

Reference computation (per batch b):
    scores = (x @ qk) @ x.T / sqrt(D)   causal-masked, softmax over keys
    out    = softmax(scores) @ x @ ov

Sharding: 2 cores per batch. Each core owns 8 of the 16 128-row query
blocks, snake-assigned ({4k, 4k+3} vs {4k+1, 4k+2}) so both halves see an
identical causal work profile -> one SPMD program, per-core data only.

Per-core pipeline (all matmuls in float32r, PSUM fp32 accumulation):
  xS = x_rows.T (PE transposes)        qT = qk.T @ xS
  xT = x_full.T (PE transposes)
  per query block i: scores strips = qT_i.T @ xT  (+additive causal mask
    from host), exp via ACT (accum_out = row-sums), PE-transpose probs
    into attnT
  per strip: PT = x_full.T-contracted attn (lhsT = natural x tiles)
  out block = (PT_i).T @ ov, scaled by 1/rowsum during PSUM evacuation.
"""

import numpy as np

import concourse.bacc as bacc
import concourse.bass as bass
import concourse.mybir as mybir
import concourse.tile as tile
from concourse.bass_interp import get_hw_module
from concourse.bass_utils import run_bass_kernel_spmd
from concourse.masks import make_identity

B, S, D = 4, 2048, 1024
NB = S // 128          # 16 row blocks per batch
NBL = NB // 2          # 8 row blocks per core
N_CORES = 8
SCALE = float(np.sqrt(D))
NEG = -1.0e30

# local block -> global block, per half (snake: exactly balanced causal work)
HALF_BLOCKS = [
    [0, 3, 4, 7, 8, 11, 12, 15],
    [1, 2, 5, 6, 9, 10, 13, 14],
]
# 512-wide score strips per local block (same for both halves)
CI = [1, 1, 2, 2, 3, 3, 4, 4]
E_STRIP = [8, 16]      # t-chunks accumulated per PT strip

F32 = mybir.dt.float32
F32R = mybir.dt.float32r


def _emit(nc, tc, x_full, x_rows, qk_in, ov_in, masks_in, y_out, ctx):
    f32r = lambda ap: ap.bitcast(F32R)
    DC = D // 128  # 8

    const = ctx.enter_context(tc.tile_pool(name="const", bufs=1))
    psA = ctx.enter_context(tc.tile_pool(name="psA", bufs=3, space="PSUM"))
    psT = ctx.enter_context(tc.tile_pool(name="psT", bufs=3, space="PSUM"))
    psP = ctx.enter_context(tc.tile_pool(name="psP", bufs=2, space="PSUM"))

    ident = const.tile([128, 128], F32, name="ident")
    make_identity(nc, ident)
    ident_r = const.tile([128, 128], F32R, name="ident_r")
    nc.vector.tensor_copy(ident_r, ident)
    # iota 0..511 along free dim; causal mask for block i's last strip is
    # (iota > thresh[:, i]) * NEG with thresh a per-core input
    iota_t = const.tile([128, 512], F32, name="iota_t")
    nc.gpsimd.iota(iota_t, pattern=[[1, 512]], base=0, channel_multiplier=0,
                   allow_small_or_imprecise_dtypes=True)
    thresh_sb = const.tile([128, NBL], F32, name="thresh_sb")
    nc.sync.dma_start(out=thresh_sb, in_=masks_in)
    zeros_sb = const.tile([128, 512], F32, name="zeros_sb")
    nc.vector.memset(zeros_sb, 0.0)
    recips = const.tile([128, NBL], F32, name="recips")

    attnT = [None, None]
    at_pool = ctx.enter_context(tc.tile_pool(name="attnT", bufs=1))
    attnT[0] = at_pool.tile([128, E_STRIP[0], 512], F32R, name="attnT0")
    attnT[1] = at_pool.tile([128, E_STRIP[1], 512], F32R, name="attnT1")
    # zero the never-written tail chunks (blocks whose causal extent ends
    # before the strip's max extent)
    for si, blkcol, lo in ((0, 0, 4), (0, 1, 4), (1, 0, 12), (1, 1, 12)):
        nc.vector.tensor_copy(
            attnT[si][:, lo:lo + 4, blkcol * 128:(blkcol + 1) * 128],
            zeros_sb.rearrange("p (a b) -> p a b", a=4),
        )

    # entered before qT/xT so its slots live below them in the pool stack:
    # the phase-3 x reload DMAs can then prefetch during phase 2 instead of
    # waiting for the xT/qT releases
    xn_p = ctx.enter_context(tc.tile_pool(name="xn_p", bufs=6))

    qT_pool = tc.tile_pool(name="qT", bufs=1)
    qTp = qT_pool.__enter__()
    qT = qTp.tile([128, DC, 1024], F32R, name="qT")

    # ---- phase 1a: xS = x_rows.T ; qT = qk.T @ xS  (one 512-row half of
    # x_rows at a time; the half-sized xS scratch is reused, the WAR dep
    # keeps PE busy with the qT matmuls in between) ----
    with tc.tile_pool(name="p1a", bufs=1) as p1a, \
         tc.tile_pool(name="xin_a", bufs=4) as xin_a:
        xS = p1a.tile([128, DC, 512], F32R, name="xS")
        qk_sb = p1a.tile([128, DC, 1024], F32R, name="qk_sb")
        qk_src = f32r(qk_in.rearrange("(c p) e -> p c e", p=128))
        xts_all = []
        for rbg in range(2):
            group = []
            for j in range(4):
                xt = xin_a.tile([128, 1024], F32R, name="xta", tag="xta")
                nc.sync.dma_start(
                    out=xt,
                    in_=f32r(x_rows[(rbg * 4 + j) * 128:(rbg * 4 + j + 1) * 128, :]))
                group.append(xt)
            xts_all.append(group)
            if rbg == 0:
                # qk chunks between the two x_rows groups in the DMA queue
                for dc in range(DC):
                    nc.sync.dma_start(
                        out=qk_sb[:, dc, :], in_=qk_src[:, dc, :])
        for rbg in range(2):
            xts = xts_all[rbg]
            for dc in range(DC):
                ps = psT.tile([128, 512], F32, name="ps_tr", tag="psT")
                for j in range(4):
                    nc.tensor.transpose(
                        ps[:, j * 128:(j + 1) * 128].bitcast(F32R),
                        xts[j][:, dc * 128:(dc + 1) * 128], ident_r)
                nc.vector.tensor_copy(xS[:, dc, :], ps)
            for ec in range(DC):
                ps = psA.tile([128, 512], F32, name="ps_qt", tag="psA")
                for dc in range(DC):
                    nc.tensor.matmul(
                        ps, qk_sb[:, dc, ec * 128:(ec + 1) * 128],
                        xS[:, dc, :],
                        start=(dc == 0), stop=(dc == DC - 1))
                nc.vector.tensor_copy(
                    qT[:, ec, rbg * 512:(rbg + 1) * 512], ps)

    # ---- phase 1b: xT = x_full.T ----
    xT_pool = tc.tile_pool(name="xT", bufs=1)
    xTp = xT_pool.__enter__()
    xT = xTp.tile([128, DC, S], F32R, name="xT")
    with tc.tile_pool(name="xin_b", bufs=6) as xin_b:
        for tbg in range(4):          # groups of 4 key blocks
            xts = []
            for j in range(4):
                xt = xin_b.tile([128, 1024], F32R, name="xtb", tag="xtb")
                nc.sync.dma_start(
                    out=xt,
                    in_=f32r(x_full[(tbg * 4 + j) * 128:(tbg * 4 + j + 1) * 128, :]))
                xts.append(xt)
            for dc in range(DC):
                ps = psT.tile([128, 512], F32, name="ps_tr", tag="psT")
                for j in range(4):
                    nc.tensor.transpose(
                        ps[:, j * 128:(j + 1) * 128].bitcast(F32R),
                        xts[j][:, dc * 128:(dc + 1) * 128], ident_r)
                nc.vector.tensor_copy(
                    xT[:, dc, tbg * 512:(tbg + 1) * 512], ps)

    # ---- phase 2: scores -> exp -> attnT, per local block ----
    with tc.tile_pool(name="p2", bufs=4) as p2, \
         tc.tile_pool(name="p2s", bufs=4) as p2s:
        for i in range(NBL):
            c = CI[i]
            si = i // 4
            blkcol = i % 4
            rs = p2s.tile([128, 4], F32, name="rs", tag="rs")
            p_strips = []
            for st in range(c):
                ps = psA.tile([128, 512], F32, name="ps_sc", tag="psA")
                for ec in range(DC):
                    nc.tensor.matmul(
                        ps, qT[:, ec, i * 128:(i + 1) * 128],
                        xT[:, ec, st * 512:(st + 1) * 512],
                        start=(ec == 0), stop=(ec == DC - 1))
                if st == c - 1:
                    mask = p2s.tile([128, 512], F32, name="mask", tag="mask",
                                    bufs=2)
                    nc.vector.tensor_scalar(
                        out=mask, in0=iota_t,
                        scalar1=thresh_sb[:, i:i + 1], scalar2=NEG,
                        op0=mybir.AluOpType.is_gt, op1=mybir.AluOpType.mult)
                    nc.vector.tensor_add(ps, ps, mask)
                p_st = p2.tile([128, 512], F32, name="p_st", tag="p_st")
                nc.scalar.activation(
                    p_st, ps,
                    mybir.ActivationFunctionType.Exp,
                    scale=1.0 / SCALE, accum_out=rs[:, st:st + 1])
                pst2 = psT.tile([128, 512], F32, name="ps_at", tag="psT")
                for j in range(4):
                    nc.tensor.transpose(
                        pst2[:, j * 128:(j + 1) * 128],
                        p_st[:, j * 128:(j + 1) * 128], ident)
                nc.vector.tensor_copy(
                    attnT[si][:, st * 4:st * 4 + 4,
                              blkcol * 128:(blkcol + 1) * 128],
                    pst2.rearrange("p (a b) -> p a b", a=4))
            rsum = p2s.tile([128, 1], F32, name="rsum", tag="rsum")
            nc.vector.reduce_sum(rsum, rs[:, 0:c], axis=mybir.AxisListType.X)
            nc.vector.reciprocal(recips[:, i:i + 1], rsum)

    xT_pool.__exit__(None, None, None)
    qT_pool.__exit__(None, None, None)

    # ---- phase 3: PT = (attn @ x).T per strip; out = PT.T @ ov ----
    with tc.tile_pool(name="p3", bufs=1) as p3, \
         tc.tile_pool(name="y_p", bufs=2) as y_p:
        ov_sb = p3.tile([128, DC, 1024], F32R, name="ov_sb")
        PT = p3.tile([128, DC, 1024], F32R, name="PT")
        # fp32 strip-scratch accumulator: PSUM can't hold 8 concurrent
        # d-chunk accumulators, so accumulate groups of 4 t-chunks in PSUM
        # and fold into SBUF
        PT32 = p3.tile([128, DC, 512], F32, name="PT32")
        ov_dc = 0  # ov is loaded per-d-chunk, interleaved with xn groups
        ov_src = f32r(ov_in.rearrange("(c p) e -> p c e", p=128))
        for si in range(2):
            E = E_STRIP[si]
            for tcg in range(E // 4):
                xns = []
                for j in range(4):
                    tc_idx = tcg * 4 + j
                    xn = xn_p.tile([128, 1024], F32R, name="xn", tag="xn")
                    nc.sync.dma_start(
                        out=xn,
                        in_=f32r(x_full[tc_idx * 128:(tc_idx + 1) * 128, :]))
                    xns.append(xn)
                while ov_dc < min(DC, 4 * (si * 2 + tcg + 1)):
                    nc.sync.dma_start(
                        out=ov_sb[:, ov_dc, :], in_=ov_src[:, ov_dc, :])
                    ov_dc += 1
                for dc in range(DC):
                    ps = psP.tile([128, 512], F32, name="ps_pt", tag="psP")
                    for j in range(4):
                        nc.tensor.matmul(
                            ps, xns[j][:, dc * 128:(dc + 1) * 128],
                            attnT[si][:, tcg * 4 + j, :],
                            start=(j == 0), stop=(j == 3))
                    if tcg == 0:
                        nc.vector.tensor_copy(PT32[:, dc, :], ps)
                    else:
                        nc.vector.tensor_add(PT32[:, dc, :], PT32[:, dc, :], ps)
            for dc in range(DC):
                nc.vector.tensor_copy(
                    PT[:, dc, si * 512:(si + 1) * 512], PT32[:, dc, :])
            for bi in range(4):
                i = si * 4 + bi
                y_sb = y_p.tile([128, 1024], F32, name="y_sb", tag="y_sb")
                for es in range(2):
                    ps = psA.tile([128, 512], F32, name="ps_o", tag="psA")
                    for dc in range(DC):
                        nc.tensor.matmul(
                            ps, PT[:, dc, i * 128:(i + 1) * 128],
                            ov_sb[:, dc, es * 512:(es + 1) * 512],
                            start=(dc == 0), stop=(dc == DC - 1))
                    nc.scalar.activation(
                        y_sb[:, es * 512:(es + 1) * 512], ps,
                        mybir.ActivationFunctionType.Copy,
                        scale=recips[:, i:i + 1])
                nc.sync.dma_start(
                    out=y_out[i * 128:(i + 1) * 128, :], in_=y_sb)


_BUILT = {}


def _build(n_reps=1, timing=False):
    """timing=True builds a variant whose big tensors are Internal DRAM
    (garbage data, tiny external IO) so per-call transfer overhead over the
    axon tunnel doesn't swamp wall-clock differencing."""
    key = (n_reps, timing)
    if key in _BUILT:
        return _BUILT[key]
    from contextlib import ExitStack

    nc = bacc.Bacc(
        "TRN2", target_bir_lowering=False, debug=False,
        enable_asserts=False, num_devices=N_CORES)
    big = dict(kind="Internal") if timing else {}
    x_full = nc.dram_tensor("x_full", [S, D], F32,
                            **(big or dict(kind="ExternalInput"))).ap()
    x_rows = nc.dram_tensor("x_rows", [S // 2, D], F32,
                            **(big or dict(kind="ExternalInput"))).ap()
    qk_in = nc.dram_tensor("qk", [D, D], F32,
                           **(big or dict(kind="ExternalInput"))).ap()
    ov_in = nc.dram_tensor("ov", [D, D], F32,
                           **(big or dict(kind="ExternalInput"))).ap()
    masks_in = nc.dram_tensor(
        "thresh", [128, NBL], F32, kind="ExternalInput").ap()
    y_out = nc.dram_tensor("y", [S // 2, D], F32,
                           **(big or dict(kind="ExternalOutput"))).ap()
    dummy_out = None
    if timing:
        dummy_out = nc.dram_tensor(
            "dummy_y", [128, 128], F32, kind="ExternalOutput").ap()

    with tile.TileContext(nc) as tc:
        if timing and n_reps > 1:
            with tc.For_i(0, n_reps, 1):
                with ExitStack() as ctx:
                    _emit(nc, tc, x_full, x_rows, qk_in, ov_in, masks_in,
                          y_out, ctx)
        else:
            for _ in range(n_reps):
                with ExitStack() as ctx:
                    _emit(nc, tc, x_full, x_rows, qk_in, ov_in, masks_in,
                          y_out, ctx)
        if timing:
            with tc.tile_pool(name="dummy_p", bufs=1) as dp:
                dt_ = dp.tile([128, 128], F32, name="dummy_sb")
                nc.sync.dma_start(out=dt_, in_=y_out[0:128, 0:128])
                nc.sync.dma_start(out=dummy_out, in_=dt_)
    nc.compile()
    nc.m = get_hw_module(nc.m)
    _BUILT[key] = nc
    return nc


def host_thresh():
    """thresh[r, i] such that last-strip column tcol is causally valid for
    row r of local block i iff tcol <= thresh[r, i]."""
    th = np.zeros((2, 128, NBL), np.float32)
    for half in range(2):
        for i, g in enumerate(HALF_BLOCKS[half]):
            th[half, :, i] = 128 * g + np.arange(128) - 512 * (CI[i] - 1)
    return th


def make_in_maps(input_data, qk, ov):
    x = np.ascontiguousarray(np.asarray(input_data, dtype=np.float32))
    qk = np.ascontiguousarray(np.asarray(qk, dtype=np.float32))
    ov = np.ascontiguousarray(np.asarray(ov, dtype=np.float32))
    th = host_thresh()
    in_maps = []
    for c in range(N_CORES):
        b, half = c // 2, c % 2
        rows = np.concatenate(
            [x[b, 128 * g:128 * (g + 1), :] for g in HALF_BLOCKS[half]], axis=0)
        in_maps.append({
            "x_full": x[b],
            "x_rows": np.ascontiguousarray(rows),
            "qk": qk,
            "ov": ov,
            "thresh": np.ascontiguousarray(th[half]),
        })
    return in_maps


def assemble(results):
    out = np.empty((B, S, D), np.float32)
    for c in range(N_CORES):
        b, half = c // 2, c % 2
        y = results[c]["y"]
        for i, g in enumerate(HALF_BLOCKS[half]):
            out[b, 128 * g:128 * (g + 1), :] = y[128 * i:128 * (i + 1), :]
    return out


def kernel(input_data, qk, ov):
    nc = _build()
    in_maps = make_in_maps(input_data, qk, ov)
    res = run_bass_kernel_spmd(nc, in_maps, core_ids=list(range(N_CORES)))
    return assemble(res.results)


# revision 35
# speedup vs baseline: 1.0905x; 1.0805x over previous
"""Causal single-head attention (B=4, S=2048, D=1024, fp32) on 8 TRN2 cores.

Reference computation (per batch b):
    scores = (x @ qk) @ x.T / sqrt(D)   causal-masked, softmax over keys
    out    = softmax(scores) @ x @ ov

Sharding: 2 cores per batch. Each core owns 8 of the 16 128-row query
blocks, snake-assigned ({4k, 4k+3} vs {4k+1, 4k+2}) so both halves see an
identical causal work profile -> one SPMD program, per-core data only.

Per-core pipeline (all matmuls in float32r, PSUM fp32 accumulation):
  xS = x_rows.T (PE transposes)        qT = qk.T @ xS
  xT = x_full.T (PE transposes)
  per query block i: scores strips = qT_i.T @ xT  (+additive causal mask
    from host), exp via ACT (accum_out = row-sums), PE-transpose probs
    into attnT
  per strip: PT = x_full.T-contracted attn (lhsT = natural x tiles)
  out block = (PT_i).T @ ov, scaled by 1/rowsum during PSUM evacuation.
"""

import numpy as np

import concourse.bacc as bacc
import concourse.bass as bass
import concourse.mybir as mybir
import concourse.tile as tile
from concourse.bass_interp import get_hw_module
from concourse.bass_utils import run_bass_kernel_spmd
from concourse.masks import make_identity

B, S, D = 4, 2048, 1024
NB = S // 128          # 16 row blocks per batch
NBL = NB // 2          # 8 row blocks per core
N_CORES = 8
SCALE = float(np.sqrt(D))
NEG = -1.0e30

# local block -> global block, per half (snake: exactly balanced causal work)
HALF_BLOCKS = [
    [0, 3, 4, 7, 8, 11, 12, 15],
    [1, 2, 5, 6, 9, 10, 13, 14],
]
# 512-wide score strips per local block (same for both halves)
CI = [1, 1, 2, 2, 3, 3, 4, 4]
E_STRIP = [8, 16]      # t-chunks accumulated per PT strip

F32 = mybir.dt.float32
F32R = mybir.dt.float32r


def _emit(nc, tc, x_full, x_rows, qk_in, ov_in, masks_in, y_out, ctx):
    f32r = lambda ap: ap.bitcast(F32R)
    DC = D // 128  # 8

    const = ctx.enter_context(tc.tile_pool(name="const", bufs=1))
    psA = ctx.enter_context(tc.tile_pool(name="psA", bufs=3, space="PSUM"))
    psT = ctx.enter_context(tc.tile_pool(name="psT", bufs=3, space="PSUM"))
    psP = ctx.enter_context(tc.tile_pool(name="psP", bufs=2, space="PSUM"))

    ident = const.tile([128, 128], F32, name="ident")
    make_identity(nc, ident)
    ident_r = const.tile([128, 128], F32R, name="ident_r")
    nc.vector.tensor_copy(ident_r, ident)
    # iota 0..511 along free dim; causal mask for block i's last strip is
    # (iota > thresh[:, i]) * NEG with thresh a per-core input
    iota_t = const.tile([128, 512], F32, name="iota_t")
    nc.gpsimd.iota(iota_t, pattern=[[1, 512]], base=0, channel_multiplier=0,
                   allow_small_or_imprecise_dtypes=True)
    thresh_sb = const.tile([128, NBL], F32, name="thresh_sb")
    nc.sync.dma_start(out=thresh_sb, in_=masks_in)
    zeros_sb = const.tile([128, 512], F32, name="zeros_sb")
    nc.vector.memset(zeros_sb, 0.0)
    recips = const.tile([128, NBL], F32, name="recips")

    attnT = [None, None]
    at_pool = ctx.enter_context(tc.tile_pool(name="attnT", bufs=1))
    attnT[0] = at_pool.tile([128, E_STRIP[0], 512], F32R, name="attnT0")
    attnT[1] = at_pool.tile([128, E_STRIP[1], 512], F32R, name="attnT1")
    # zero the never-written tail chunks (blocks whose causal extent ends
    # before the strip's max extent)
    for si, blkcol, lo in ((0, 0, 4), (0, 1, 4), (1, 0, 12), (1, 1, 12)):
        nc.vector.tensor_copy(
            attnT[si][:, lo:lo + 4, blkcol * 128:(blkcol + 1) * 128],
            zeros_sb.rearrange("p (a b) -> p a b", a=4),
        )

    # entered before qT/xT so its slots live below them in the pool stack:
    # the phase-3 x reload DMAs can then prefetch during phase 2 instead of
    # waiting for the xT/qT releases
    xn_p = ctx.enter_context(tc.tile_pool(name="xn_p", bufs=6))

    qT_pool = tc.tile_pool(name="qT", bufs=1)
    qTp = qT_pool.__enter__()
    qT = qTp.tile([128, DC, 1024], F32R, name="qT")

    # ---- phase 1a: xS = x_rows.T ; qT = qk.T @ xS  (one 512-row half of
    # x_rows at a time; the half-sized xS scratch is reused, the WAR dep
    # keeps PE busy with the qT matmuls in between) ----
    with tc.tile_pool(name="p1a", bufs=1) as p1a, \
         tc.tile_pool(name="xin_a", bufs=4) as xin_a:
        xS = p1a.tile([128, DC, 512], F32R, name="xS")
        qk_sb = p1a.tile([128, DC, 1024], F32R, name="qk_sb")
        qk_src = f32r(qk_in.rearrange("(c p) e -> p c e", p=128))
        xts_all = []
        for rbg in range(2):
            group = []
            for j in range(4):
                xt = xin_a.tile([128, 1024], F32R, name="xta", tag="xta")
                nc.sync.dma_start(
                    out=xt,
                    in_=f32r(x_rows[(rbg * 4 + j) * 128:(rbg * 4 + j + 1) * 128, :]))
                group.append(xt)
            xts_all.append(group)
            if rbg == 0:
                # qk chunks between the two x_rows groups in the DMA queue
                for dc in range(DC):
                    nc.sync.dma_start(
                        out=qk_sb[:, dc, :], in_=qk_src[:, dc, :])
        for rbg in range(2):
            xts = xts_all[rbg]
            for dc in range(DC):
                ps = psT.tile([128, 512], F32, name="ps_tr", tag="psT")
                for j in range(4):
                    nc.tensor.transpose(
                        ps[:, j * 128:(j + 1) * 128].bitcast(F32R),
                        xts[j][:, dc * 128:(dc + 1) * 128], ident_r)
                nc.vector.tensor_copy(xS[:, dc, :], ps)
            for ec in range(DC):
                ps = psA.tile([128, 512], F32, name="ps_qt", tag="psA")
                for dc in range(DC):
                    nc.tensor.matmul(
                        ps, qk_sb[:, dc, ec * 128:(ec + 1) * 128],
                        xS[:, dc, :],
                        start=(dc == 0), stop=(dc == DC - 1))
                nc.vector.tensor_copy(
                    qT[:, ec, rbg * 512:(rbg + 1) * 512], ps)

    # ---- phase 1b: xT = x_full.T ----
    xT_pool = tc.tile_pool(name="xT", bufs=1)
    xTp = xT_pool.__enter__()
    xT = xTp.tile([128, DC, S], F32R, name="xT")
    with tc.tile_pool(name="xin_b", bufs=8) as xin_b:
        for tbg in range(4):          # groups of 4 key blocks
            xts = []
            for j in range(4):
                xt = xin_b.tile([128, 1024], F32R, name="xtb", tag="xtb")
                nc.sync.dma_start(
                    out=xt,
                    in_=f32r(x_full[(tbg * 4 + j) * 128:(tbg * 4 + j + 1) * 128, :]))
                xts.append(xt)
            for dc in range(DC):
                ps = psT.tile([128, 512], F32, name="ps_tr", tag="psT")
                for j in range(4):
                    nc.tensor.transpose(
                        ps[:, j * 128:(j + 1) * 128].bitcast(F32R),
                        xts[j][:, dc * 128:(dc + 1) * 128], ident_r)
                nc.vector.tensor_copy(
                    xT[:, dc, tbg * 512:(tbg + 1) * 512], ps)

    # ---- phase 2: scores -> exp -> attnT, per local block ----
    with tc.tile_pool(name="p2", bufs=4) as p2, \
         tc.tile_pool(name="p2s", bufs=4) as p2s:
        for i in range(NBL):
            c = CI[i]
            si = i // 4
            blkcol = i % 4
            rs = p2s.tile([128, 4], F32, name="rs", tag="rs")
            p_strips = []
            for st in range(c):
                ps = psA.tile([128, 512], F32, name="ps_sc", tag="psA")
                for ec in range(DC):
                    nc.tensor.matmul(
                        ps, qT[:, ec, i * 128:(i + 1) * 128],
                        xT[:, ec, st * 512:(st + 1) * 512],
                        start=(ec == 0), stop=(ec == DC - 1))
                if st == c - 1:
                    mask = p2s.tile([128, 512], F32, name="mask", tag="mask",
                                    bufs=2)
                    nc.vector.tensor_scalar(
                        out=mask, in0=iota_t,
                        scalar1=thresh_sb[:, i:i + 1], scalar2=NEG,
                        op0=mybir.AluOpType.is_gt, op1=mybir.AluOpType.mult)
                    nc.vector.tensor_add(ps, ps, mask)
                p_st = p2.tile([128, 512], F32R, name="p_st", tag="p_st")
                nc.scalar.activation(
                    p_st, ps,
                    mybir.ActivationFunctionType.Exp,
                    scale=1.0 / SCALE, accum_out=rs[:, st:st + 1])
                pst2 = psT.tile([128, 512], F32, name="ps_at", tag="psT")
                for j in range(4):
                    nc.tensor.transpose(
                        pst2[:, j * 128:(j + 1) * 128].bitcast(F32R),
                        p_st[:, j * 128:(j + 1) * 128], ident_r)
                nc.vector.tensor_copy(
                    attnT[si][:, st * 4:st * 4 + 4,
                              blkcol * 128:(blkcol + 1) * 128],
                    pst2.rearrange("p (a b) -> p a b", a=4))
            rsum = p2s.tile([128, 1], F32, name="rsum", tag="rsum")
            nc.vector.reduce_sum(rsum, rs[:, 0:c], axis=mybir.AxisListType.X)
            nc.vector.reciprocal(recips[:, i:i + 1], rsum)

    xT_pool.__exit__(None, None, None)
    qT_pool.__exit__(None, None, None)

    # ---- phase 3: PT = (attn @ x).T per strip; out = PT.T @ ov ----
    with tc.tile_pool(name="p3", bufs=1) as p3, \
         tc.tile_pool(name="y_p", bufs=2) as y_p:
        ov_sb = p3.tile([128, DC, 1024], F32R, name="ov_sb")
        PT = p3.tile([128, DC, 1024], F32R, name="PT")
        # fp32 strip-scratch accumulator: PSUM can't hold 8 concurrent
        # d-chunk accumulators, so accumulate groups of 4 t-chunks in PSUM
        # and fold into SBUF
        PT32 = p3.tile([128, DC, 512], F32, name="PT32")
        ov_dc = 0  # ov is loaded per-d-chunk, interleaved with xn groups
        ov_src = f32r(ov_in.rearrange("(c p) e -> p c e", p=128))
        for si in range(2):
            E = E_STRIP[si]
            for tcg in range(E // 4):
                xns = []
                for j in range(4):
                    tc_idx = tcg * 4 + j
                    xn = xn_p.tile([128, 1024], F32R, name="xn", tag="xn")
                    nc.sync.dma_start(
                        out=xn,
                        in_=f32r(x_full[tc_idx * 128:(tc_idx + 1) * 128, :]))
                    xns.append(xn)
                while ov_dc < min(DC, 4 * (si * 2 + tcg + 1)):
                    nc.sync.dma_start(
                        out=ov_sb[:, ov_dc, :], in_=ov_src[:, ov_dc, :])
                    ov_dc += 1
                for dc in range(DC):
                    ps = psP.tile([128, 512], F32, name="ps_pt", tag="psP")
                    for j in range(4):
                        nc.tensor.matmul(
                            ps, xns[j][:, dc * 128:(dc + 1) * 128],
                            attnT[si][:, tcg * 4 + j, :],
                            start=(j == 0), stop=(j == 3))
                    if tcg == 0:
                        nc.vector.tensor_copy(PT32[:, dc, :], ps)
                    else:
                        nc.vector.tensor_add(PT32[:, dc, :], PT32[:, dc, :], ps)
            for dc in range(DC):
                nc.vector.tensor_copy(
                    PT[:, dc, si * 512:(si + 1) * 512], PT32[:, dc, :])
            for bi in range(4):
                i = si * 4 + bi
                y_sb = y_p.tile([128, 1024], F32, name="y_sb", tag="y_sb")
                for es in range(2):
                    ps = psA.tile([128, 512], F32, name="ps_o", tag="psA")
                    for dc in range(DC):
                        nc.tensor.matmul(
                            ps, PT[:, dc, i * 128:(i + 1) * 128],
                            ov_sb[:, dc, es * 512:(es + 1) * 512],
                            start=(dc == 0), stop=(dc == DC - 1))
                    nc.scalar.activation(
                        y_sb[:, es * 512:(es + 1) * 512], ps,
                        mybir.ActivationFunctionType.Copy,
                        scale=recips[:, i:i + 1])
                nc.sync.dma_start(
                    out=y_out[i * 128:(i + 1) * 128, :], in_=y_sb)


_BUILT = {}


def _build(n_reps=1, timing=False):
    """timing=True builds a variant whose big tensors are Internal DRAM
    (garbage data, tiny external IO) so per-call transfer overhead over the
    axon tunnel doesn't swamp wall-clock differencing."""
    key = (n_reps, timing)
    if key in _BUILT:
        return _BUILT[key]
    from contextlib import ExitStack

    nc = bacc.Bacc(
        "TRN2", target_bir_lowering=False, debug=False,
        enable_asserts=False, num_devices=N_CORES)
    big = dict(kind="Internal") if timing else {}
    x_full = nc.dram_tensor("x_full", [S, D], F32,
                            **(big or dict(kind="ExternalInput"))).ap()
    x_rows = nc.dram_tensor("x_rows", [S // 2, D], F32,
                            **(big or dict(kind="ExternalInput"))).ap()
    qk_in = nc.dram_tensor("qk", [D, D], F32,
                           **(big or dict(kind="ExternalInput"))).ap()
    ov_in = nc.dram_tensor("ov", [D, D], F32,
                           **(big or dict(kind="ExternalInput"))).ap()
    masks_in = nc.dram_tensor(
        "thresh", [128, NBL], F32, kind="ExternalInput").ap()
    y_out = nc.dram_tensor("y", [S // 2, D], F32,
                           **(big or dict(kind="ExternalOutput"))).ap()
    dummy_out = None
    if timing:
        dummy_out = nc.dram_tensor(
            "dummy_y", [128, 128], F32, kind="ExternalOutput").ap()

    with tile.TileContext(nc) as tc:
        if timing and n_reps > 1:
            with tc.For_i(0, n_reps, 1):
                with ExitStack() as ctx:
                    _emit(nc, tc, x_full, x_rows, qk_in, ov_in, masks_in,
                          y_out, ctx)
        else:
            for _ in range(n_reps):
                with ExitStack() as ctx:
                    _emit(nc, tc, x_full, x_rows, qk_in, ov_in, masks_in,
                          y_out, ctx)
        if timing:
            with tc.tile_pool(name="dummy_p", bufs=1) as dp:
                dt_ = dp.tile([128, 128], F32, name="dummy_sb")
                nc.sync.dma_start(out=dt_, in_=y_out[0:128, 0:128])
                nc.sync.dma_start(out=dummy_out, in_=dt_)
    nc.compile()
    nc.m = get_hw_module(nc.m)
    _BUILT[key] = nc
    return nc


def host_thresh():
    """thresh[r, i] such that last-strip column tcol is causally valid for
    row r of local block i iff tcol <= thresh[r, i]."""
    th = np.zeros((2, 128, NBL), np.float32)
    for half in range(2):
        for i, g in enumerate(HALF_BLOCKS[half]):
            th[half, :, i] = 128 * g + np.arange(128) - 512 * (CI[i] - 1)
    return th


def make_in_maps(input_data, qk, ov):
    x = np.ascontiguousarray(np.asarray(input_data, dtype=np.float32))
    qk = np.ascontiguousarray(np.asarray(qk, dtype=np.float32))
    ov = np.ascontiguousarray(np.asarray(ov, dtype=np.float32))
    th = host_thresh()
    in_maps = []
    for c in range(N_CORES):
        b, half = c // 2, c % 2
        rows = np.concatenate(
            [x[b, 128 * g:128 * (g + 1), :] for g in HALF_BLOCKS[half]], axis=0)
        in_maps.append({
            "x_full": x[b],
            "x_rows": np.ascontiguousarray(rows),
            "qk": qk,
            "ov": ov,
            "thresh": np.ascontiguousarray(th[half]),
        })
    return in_maps


def assemble(results):
    out = np.empty((B, S, D), np.float32)
    for c in range(N_CORES):
        b, half = c // 2, c % 2
        y = results[c]["y"]
        for i, g in enumerate(HALF_BLOCKS[half]):
            out[b, 128 * g:128 * (g + 1), :] = y[128 * i:128 * (i + 1), :]
    return out


def kernel(input_data, qk, ov):
    nc = _build()
    in_maps = make_in_maps(input_data, qk, ov)
    res = run_bass_kernel_spmd(nc, in_maps, core_ids=list(range(N_CORES)))
    return assemble(res.results)


# revision 36
# speedup vs baseline: 1.4046x; 1.2881x over previous
"""Causal single-head attention (B=4, S=2048, D=1024, fp32) on 8 TRN2 cores.

Reference computation (per batch b):
    scores = (x @ qk) @ x.T / sqrt(D)   causal-masked, softmax over keys
    out    = softmax(scores) @ x @ ov

Sharding: 2 cores per batch. Each core owns 8 of the 16 128-row query
blocks, snake-assigned ({4k, 4k+3} vs {4k+1, 4k+2}) so both halves see an
identical causal work profile -> one SPMD program, per-core data only.

Per-core pipeline (all matmuls/transposes in float32r = full-rate PE,
PSUM fp32 accumulation):
  xS = x_rows.T (PE transposes, one 512-half at a time); qT = qk.T @ xS
  xT = x_full.T (PE transposes)
  per query block i: score strips = qT_i.T @ xT; additive causal mask
    built on-chip from iota vs a per-core threshold input; exp on ACT
    (accum_out gives row-sums for free); PE-transpose probs into attnT
  per 512-col strip of local rows: PT = sum_t x[t,:]^T attnT[t,:]
    (lhsT = natural x tiles streamed from DRAM, 4-chunk PSUM groups
    folded into an fp32 SBUF accumulator)
  out block = (PT_i).T @ ov, scaled by 1/rowsum during the ACT PSUM
    evacuation; rows written back compact, host re-scatters.
"""

import numpy as np

import concourse.bacc as bacc
import concourse.mybir as mybir
import concourse.tile as tile
from concourse.bass_interp import get_hw_module
from concourse.bass_utils import run_bass_kernel_spmd
from concourse.masks import make_identity

B, S, D = 4, 2048, 1024
NB = S // 128          # 16 row blocks per batch
NBL = NB // 2          # 8 row blocks per core
N_CORES = 8
SCALE = float(np.sqrt(D))
NEG = -1.0e30

# local block -> global block, per half (snake: exactly balanced causal work)
HALF_BLOCKS = [
    [0, 3, 4, 7, 8, 11, 12, 15],
    [1, 2, 5, 6, 9, 10, 13, 14],
]
# 512-wide score strips per local block (same for both halves)
CI = [1, 1, 2, 2, 3, 3, 4, 4]
E_STRIP = [8, 16]      # t-chunks accumulated per PT strip

F32 = mybir.dt.float32
F32R = mybir.dt.float32r


def _emit(nc, tc, x_full, x_rows, qk_in, ov_in, masks_in, y_out, ctx):
    f32r = lambda ap: ap.bitcast(F32R)
    DC = D // 128  # 8

    const = ctx.enter_context(tc.tile_pool(name="const", bufs=1))
    psA = ctx.enter_context(tc.tile_pool(name="psA", bufs=3, space="PSUM"))
    psT = ctx.enter_context(tc.tile_pool(name="psT", bufs=3, space="PSUM"))
    psP = ctx.enter_context(tc.tile_pool(name="psP", bufs=2, space="PSUM"))

    ident = const.tile([128, 128], F32, name="ident")
    make_identity(nc, ident)
    ident_r = const.tile([128, 128], F32R, name="ident_r")
    nc.vector.tensor_copy(ident_r, ident)
    # iota 0..511 along free dim; causal mask for block i's last strip is
    # (iota > thresh[:, i]) * NEG with thresh a per-core input
    iota_t = const.tile([128, 512], F32, name="iota_t")
    nc.gpsimd.iota(iota_t, pattern=[[1, 512]], base=0, channel_multiplier=0,
                   allow_small_or_imprecise_dtypes=True)
    thresh_sb = const.tile([128, NBL], F32, name="thresh_sb")
    nc.sync.dma_start(out=thresh_sb, in_=masks_in)
    zeros_sb = const.tile([128, 512], F32, name="zeros_sb")
    nc.vector.memset(zeros_sb, 0.0)
    recips = const.tile([128, NBL], F32, name="recips")

    attnT = [None, None]
    at_pool = ctx.enter_context(tc.tile_pool(name="attnT", bufs=1))
    attnT[0] = at_pool.tile([128, E_STRIP[0], 512], F32R, name="attnT0")
    attnT[1] = at_pool.tile([128, E_STRIP[1], 512], F32R, name="attnT1")
    # zero the never-written tail chunks (blocks whose causal extent ends
    # before the strip's max extent)
    for si, blkcol, lo in ((0, 0, 4), (0, 1, 4), (1, 0, 12), (1, 1, 12)):
        nc.vector.tensor_copy(
            attnT[si][:, lo:lo + 4, blkcol * 128:(blkcol + 1) * 128],
            zeros_sb.rearrange("p (a b) -> p a b", a=4),
        )

    # entered before qT/xT so its slots live below them in the pool stack:
    # the phase-3 x reload DMAs can then prefetch during phase 2 instead of
    # waiting for the xT/qT releases
    xn_p = ctx.enter_context(tc.tile_pool(name="xn_p", bufs=6))

    qT_pool = tc.tile_pool(name="qT", bufs=1)
    qTp = qT_pool.__enter__()
    qT = qTp.tile([128, DC, 1024], F32R, name="qT")

    # ---- phase 1a: xS = x_rows.T ; qT = qk.T @ xS  (one 512-row half of
    # x_rows at a time; the half-sized xS scratch is reused, the WAR dep
    # keeps PE busy with the qT matmuls in between) ----
    with tc.tile_pool(name="p1a", bufs=1) as p1a, \
         tc.tile_pool(name="xin_a", bufs=4) as xin_a:
        xS = p1a.tile([128, DC, 512], F32R, name="xS")
        qk_sb = p1a.tile([128, DC, 1024], F32R, name="qk_sb")
        qk_src = f32r(qk_in.rearrange("(c p) e -> p c e", p=128))
        xts_all = []
        for rbg in range(2):
            group = []
            for j in range(4):
                xt = xin_a.tile([128, 1024], F32R, name="xta", tag="xta")
                nc.sync.dma_start(
                    out=xt,
                    in_=f32r(x_rows[(rbg * 4 + j) * 128:(rbg * 4 + j + 1) * 128, :]))
                group.append(xt)
            xts_all.append(group)
            if rbg == 0:
                # qk chunks between the two x_rows groups in the DMA queue
                for dc in range(DC):
                    nc.sync.dma_start(
                        out=qk_sb[:, dc, :], in_=qk_src[:, dc, :])
        for rbg in range(2):
            xts = xts_all[rbg]
            for dc in range(DC):
                ps = psT.tile([128, 512], F32, name="ps_tr", tag="psT")
                for j in range(4):
                    nc.tensor.transpose(
                        ps[:, j * 128:(j + 1) * 128].bitcast(F32R),
                        xts[j][:, dc * 128:(dc + 1) * 128], ident_r)
                nc.vector.tensor_copy(xS[:, dc, :], ps)
            for ec in range(DC):
                ps = psA.tile([128, 512], F32, name="ps_qt", tag="psA")
                for dc in range(DC):
                    nc.tensor.matmul(
                        ps, qk_sb[:, dc, ec * 128:(ec + 1) * 128],
                        xS[:, dc, :],
                        start=(dc == 0), stop=(dc == DC - 1))
                nc.vector.tensor_copy(
                    qT[:, ec, rbg * 512:(rbg + 1) * 512], ps)

    # ---- phase 1b: xT = x_full.T ----
    xT_pool = tc.tile_pool(name="xT", bufs=1)
    xTp = xT_pool.__enter__()
    xT = xTp.tile([128, DC, S], F32R, name="xT")
    with tc.tile_pool(name="xin_b", bufs=8) as xin_b:
        for tbg in range(4):          # groups of 4 key blocks
            xts = []
            for j in range(4):
                xt = xin_b.tile([128, 1024], F32R, name="xtb", tag="xtb")
                nc.sync.dma_start(
                    out=xt,
                    in_=f32r(x_full[(tbg * 4 + j) * 128:(tbg * 4 + j + 1) * 128, :]))
                xts.append(xt)
            for dc in range(DC):
                ps = psT.tile([128, 512], F32, name="ps_tr", tag="psT")
                for j in range(4):
                    nc.tensor.transpose(
                        ps[:, j * 128:(j + 1) * 128].bitcast(F32R),
                        xts[j][:, dc * 128:(dc + 1) * 128], ident_r)
                nc.vector.tensor_copy(
                    xT[:, dc, tbg * 512:(tbg + 1) * 512], ps)

    # ---- phase 2: scores -> exp -> attnT, per local block ----
    with tc.tile_pool(name="p2", bufs=4) as p2, \
         tc.tile_pool(name="p2s", bufs=4) as p2s:
        for i in range(NBL):
            c = CI[i]
            si = i // 4
            blkcol = i % 4
            rs = p2s.tile([128, 4], F32, name="rs", tag="rs")
            for st in range(c):
                ps = psA.tile([128, 512], F32, name="ps_sc", tag="psA")
                for ec in range(DC):
                    nc.tensor.matmul(
                        ps, qT[:, ec, i * 128:(i + 1) * 128],
                        xT[:, ec, st * 512:(st + 1) * 512],
                        start=(ec == 0), stop=(ec == DC - 1))
                if st == c - 1:
                    mask = p2s.tile([128, 512], F32, name="mask", tag="mask",
                                    bufs=2)
                    nc.vector.tensor_scalar(
                        out=mask, in0=iota_t,
                        scalar1=thresh_sb[:, i:i + 1], scalar2=NEG,
                        op0=mybir.AluOpType.is_gt, op1=mybir.AluOpType.mult)
                    nc.vector.tensor_add(ps, ps, mask)
                p_st = p2.tile([128, 512], F32R, name="p_st", tag="p_st")
                nc.scalar.activation(
                    p_st, ps,
                    mybir.ActivationFunctionType.Exp,
                    scale=1.0 / SCALE, accum_out=rs[:, st:st + 1])
                pst2 = psT.tile([128, 512], F32, name="ps_at", tag="psT")
                for j in range(4):
                    nc.tensor.transpose(
                        pst2[:, j * 128:(j + 1) * 128].bitcast(F32R),
                        p_st[:, j * 128:(j + 1) * 128], ident_r)
                nc.vector.tensor_copy(
                    attnT[si][:, st * 4:st * 4 + 4,
                              blkcol * 128:(blkcol + 1) * 128],
                    pst2.rearrange("p (a b) -> p a b", a=4))
            rsum = p2s.tile([128, 1], F32, name="rsum", tag="rsum")
            nc.vector.reduce_sum(rsum, rs[:, 0:c], axis=mybir.AxisListType.X)
            nc.vector.reciprocal(recips[:, i:i + 1], rsum)

    xT_pool.__exit__(None, None, None)
    qT_pool.__exit__(None, None, None)

    # ---- phase 3: PT = (attn @ x).T per strip; out = PT.T @ ov ----
    with tc.tile_pool(name="p3", bufs=1) as p3, \
         tc.tile_pool(name="y_p", bufs=2) as y_p:
        ov_sb = p3.tile([128, DC, 1024], F32R, name="ov_sb")
        PT = p3.tile([128, DC, 1024], F32R, name="PT")
        # fp32 strip-scratch accumulator: PSUM can't hold 8 concurrent
        # d-chunk accumulators, so accumulate groups of 4 t-chunks in PSUM
        # and fold into SBUF
        PT32 = p3.tile([128, DC, 512], F32, name="PT32")
        ov_dc = 0  # ov is loaded per-d-chunk, interleaved with xn groups
        ov_src = f32r(ov_in.rearrange("(c p) e -> p c e", p=128))
        for si in range(2):
            E = E_STRIP[si]
            for tcg in range(E // 4):
                xns = []
                for j in range(4):
                    tc_idx = tcg * 4 + j
                    xn = xn_p.tile([128, 1024], F32R, name="xn", tag="xn")
                    nc.sync.dma_start(
                        out=xn,
                        in_=f32r(x_full[tc_idx * 128:(tc_idx + 1) * 128, :]))
                    xns.append(xn)
                while ov_dc < min(DC, 4 * (si * 2 + tcg + 1)):
                    nc.sync.dma_start(
                        out=ov_sb[:, ov_dc, :], in_=ov_src[:, ov_dc, :])
                    ov_dc += 1
                for dc in range(DC):
                    ps = psP.tile([128, 512], F32, name="ps_pt", tag="psP")
                    for j in range(4):
                        nc.tensor.matmul(
                            ps, xns[j][:, dc * 128:(dc + 1) * 128],
                            attnT[si][:, tcg * 4 + j, :],
                            start=(j == 0), stop=(j == 3))
                    if tcg == 0:
                        nc.vector.tensor_copy(PT32[:, dc, :], ps)
                    else:
                        nc.vector.tensor_add(PT32[:, dc, :], PT32[:, dc, :], ps)
            for dc in range(DC):
                nc.vector.tensor_copy(
                    PT[:, dc, si * 512:(si + 1) * 512], PT32[:, dc, :])
            for bi in range(4):
                i = si * 4 + bi
                y_sb = y_p.tile([128, 1024], F32, name="y_sb", tag="y_sb")
                for es in range(2):
                    ps = psA.tile([128, 512], F32, name="ps_o", tag="psA")
                    for dc in range(DC):
                        nc.tensor.matmul(
                            ps, PT[:, dc, i * 128:(i + 1) * 128],
                            ov_sb[:, dc, es * 512:(es + 1) * 512],
                            start=(dc == 0), stop=(dc == DC - 1))
                    nc.scalar.activation(
                        y_sb[:, es * 512:(es + 1) * 512], ps,
                        mybir.ActivationFunctionType.Copy,
                        scale=recips[:, i:i + 1])
                nc.sync.dma_start(
                    out=y_out[i * 128:(i + 1) * 128, :], in_=y_sb)


_BUILT = {}


def _build(n_reps=1, timing=False):
    """timing=True builds a variant whose big tensors are Internal DRAM
    (garbage data, tiny external IO) so per-call transfer overhead over the
    axon tunnel doesn't swamp wall-clock differencing."""
    key = (n_reps, timing)
    if key in _BUILT:
        return _BUILT[key]
    from contextlib import ExitStack

    nc = bacc.Bacc(
        "TRN2", target_bir_lowering=False, debug=False,
        enable_asserts=False, num_devices=N_CORES)
    big = dict(kind="Internal") if timing else {}
    x_full = nc.dram_tensor("x_full", [S, D], F32,
                            **(big or dict(kind="ExternalInput"))).ap()
    x_rows = nc.dram_tensor("x_rows", [S // 2, D], F32,
                            **(big or dict(kind="ExternalInput"))).ap()
    qk_in = nc.dram_tensor("qk", [D, D], F32,
                           **(big or dict(kind="ExternalInput"))).ap()
    ov_in = nc.dram_tensor("ov", [D, D], F32,
                           **(big or dict(kind="ExternalInput"))).ap()
    masks_in = nc.dram_tensor(
        "thresh", [128, NBL], F32, kind="ExternalInput").ap()
    y_out = nc.dram_tensor("y", [S // 2, D], F32,
                           **(big or dict(kind="ExternalOutput"))).ap()
    dummy_out = None
    if timing:
        dummy_out = nc.dram_tensor(
            "dummy_y", [128, 128], F32, kind="ExternalOutput").ap()

    with tile.TileContext(nc) as tc:
        if timing and n_reps > 1:
            with tc.For_i(0, n_reps, 1):
                with ExitStack() as ctx:
                    _emit(nc, tc, x_full, x_rows, qk_in, ov_in, masks_in,
                          y_out, ctx)
        else:
            for _ in range(n_reps):
                with ExitStack() as ctx:
                    _emit(nc, tc, x_full, x_rows, qk_in, ov_in, masks_in,
                          y_out, ctx)
        if timing:
            with tc.tile_pool(name="dummy_p", bufs=1) as dp:
                dt_ = dp.tile([128, 128], F32, name="dummy_sb")
                nc.sync.dma_start(out=dt_, in_=y_out[0:128, 0:128])
                nc.sync.dma_start(out=dummy_out, in_=dt_)
    nc.compile()
    nc.m = get_hw_module(nc.m)
    _BUILT[key] = nc
    return nc


def host_thresh():
    """thresh[r, i] such that last-strip column tcol is causally valid for
    row r of local block i iff tcol <= thresh[r, i]."""
    th = np.zeros((2, 128, NBL), np.float32)
    for half in range(2):
        for i, g in enumerate(HALF_BLOCKS[half]):
            th[half, :, i] = 128 * g + np.arange(128) - 512 * (CI[i] - 1)
    return th


def make_in_maps(input_data, qk, ov):
    x = np.ascontiguousarray(np.asarray(input_data, dtype=np.float32))
    qk = np.ascontiguousarray(np.asarray(qk, dtype=np.float32))
    ov = np.ascontiguousarray(np.asarray(ov, dtype=np.float32))
    th = host_thresh()
    in_maps = []
    for c in range(N_CORES):
        b, half = c // 2, c % 2
        rows = np.concatenate(
            [x[b, 128 * g:128 * (g + 1), :] for g in HALF_BLOCKS[half]], axis=0)
        in_maps.append({
            "x_full": x[b],
            "x_rows": np.ascontiguousarray(rows),
            "qk": qk,
            "ov": ov,
            "thresh": np.ascontiguousarray(th[half]),
        })
    return in_maps


def assemble(results):
    out = np.empty((B, S, D), np.float32)
    for c in range(N_CORES):
        b, half = c // 2, c % 2
        y = results[c]["y"]
        for i, g in enumerate(HALF_BLOCKS[half]):
            out[b, 128 * g:128 * (g + 1), :] = y[128 * i:128 * (i + 1), :]
    return out


def kernel(input_data, qk, ov):
    nc = _build()
    in_maps = make_in_maps(input_data, qk, ov)
    res = run_bass_kernel_spmd(nc, in_maps, core_ids=list(range(N_CORES)))
    return assemble(res.results)
